# revision 8
# baseline (speedup 1.0000x reference)
"""Trainium2 Bass kernel: LADIES mini-batch ER-GCN (2-layer relational GCN).

Contract: kernel(**inputs) takes the FULL unsharded inputs (numpy, keyed as in
setup_inputs) and returns the FULL [256, 32] float32 output.

Strategy (8 NeuronCores, relation-sharded layer 1, output-row-sharded layer 2):
  - h1 = relu(A0 @ xw + b1) dominates: A0 is [1024, 131072] f32 = 512 MB.
    Core c owns relations {2c, 2c+1} = a contiguous 64 MB column block of A0,
    host-packed into the exact [128-partition, k-tile, n1] order the kernel
    consumes and streamed as bf16 (32 MB/core): every 1 MB DMA is 8 KB-
    contiguous per partition (full HBM rate).
  - The tiny per-relation xw = x @ w1[r] slices (2 MB/core, the same bytes as
    shipping x itself) are precomputed on the host per the sharding hint
    ("split A_0 column-blocks and the matching relation slices of xw"), so
    the PE only runs the A0-stream matmuls and never stalls on an
    xt -> xw -> copy chain.
  - A dep-free warm-up matmul block at t~0 lifts HAM out of the K=4/8
    half-rate state before the stream begins; an anchored second block keeps
    the PE warm across the AllReduce wait so the layer-2 matmuls run warm.
  - ONE AllReduce of the full h1 partial [64, 1024] in bf16 (128 KB).  A
    dep-free dummy 1-column AllReduce issued at t~0 absorbs the ~11.5 us
    first-collective setup and queues behind the runtime's entry barrier, so
    the real AR starts ~1 us after its doorbell.
  - Layer 2 runs fully in bf16: after the AR every core has full h1; core c
    computes out.T[:, 32c:32c+32] (its 32 output rows) against a host-packed
    A1.T column block -- no second collective; the host concatenates slices.
"""

import numpy as np
import ml_dtypes

# Problem dimensions (fixed by the problem spec).
R, NB = 16, 16
N2, N1, NOUT = 8192, 1024, 256
F, E, C = 128, 64, 32

NCORES = 8
RPC = R // NCORES            # relations per core = 2
KPC = RPC * N2               # layer-1 contraction rows per core = 16384
NKT = KPC // 128             # k-tiles per core = 128
NB2 = N2 // 128              # n2-blocks per relation = 64
NB1 = N1 // 128              # n1-blocks = 8
KT_PER_DMA = 4               # k-tiles per A0 DMA (1 MB bf16 transfers)
NG = NKT // KT_PER_DMA       # A0 DMA groups = 32
NOPC = NOUT // NCORES        # output rows per core = 32
WARM_MMS = 16                # dep-free PE warm-up matmuls (~7 us @ K=4/8)
WARM2_MMS = 90               # keep-warm matmuls across the AllReduce wait

_cache = {}
last_results = None          # BassKernelResults from the most recent run


def _build_module(repeats=1, use_collectives=True):
    import concourse.bacc as bacc
    import concourse.tile as tile
    import concourse.tile as tile_mod
    import concourse.mybir as mybir

    f32 = mybir.dt.float32
    bf16 = mybir.dt.bfloat16

    nc = bacc.Bacc("TRN2", target_bir_lowering=False, debug=False,
                   num_devices=NCORES)

    xwt = nc.dram_tensor("xwt", [128, NKT * E], bf16, kind="ExternalInput")
    a0t = nc.dram_tensor("a0t", [128, NKT * N1], bf16, kind="ExternalInput")
    a1t = nc.dram_tensor("a1t", [128, NKT * C], bf16, kind="ExternalInput")
    w2a = nc.dram_tensor("w2a", [E, R * C], bf16, kind="ExternalInput")
    b1 = nc.dram_tensor("b1", [E, 1], f32, kind="ExternalInput")
    b2 = nc.dram_tensor("b2", [C, 1], f32, kind="ExternalInput")
    outT = nc.dram_tensor("outT", [C, NOPC], f32, kind="ExternalOutput")

    rg = [list(range(NCORES))]

    with tile.TileContext(nc) as tc:
        with (
            tc.tile_pool(name="const", bufs=1) as constp,
            tc.tile_pool(name="xwp", bufs=1) as xwp,
            tc.tile_pool(name="a0p", bufs=10) as a0p,
            tc.tile_pool(name="a1p", bufs=1) as a1p,
            tc.tile_pool(name="h1p", bufs=2) as h1p,
            tc.tile_pool(name="h2p", bufs=8) as h2p,
            tc.tile_pool(name="pswu", bufs=1, space="PSUM") as pswu,
            tc.tile_pool(name="psh1", bufs=1, space="PSUM") as psh1,
            tc.tile_pool(name="psh2", bufs=3, space="PSUM") as psh2,
            tc.tile_pool(name="psout", bufs=1, space="PSUM") as psoutp,
            tc.tile_pool(name="dram", bufs=1, space="DRAM") as dramp,
        ):
            # ---- PE warm-up: dep-free back-to-back matmuls so HAM reaches
            # K=8/8 before the stream matmuls begin.
            wu_sb = constp.tile([128, 512], bf16, name="wu_sb")
            nc.vector.memset(wu_sb[:], 0.0)
            ps_wu = pswu.tile([128, 512], f32, name="ps_wu", tag="ps_wu")
            for i in range(WARM_MMS):
                nc.tensor.matmul(
                    ps_wu[:], wu_sb[:, :128], wu_sb[:],
                    start=(i == 0), stop=(i == WARM_MMS - 1),
                )

            # ---- dummy first collective: absorbs the ~11.5 us first-AR
            # ncfw setup + entry barrier off the critical path.
            cc_din = dramp.tile([E, 1], bf16, name="cc_din")
            cc_dout = dramp.tile([E, 1], bf16, name="cc_dout",
                                 addr_space="Shared")
            nc.gpsimd.dma_start(cc_din[:], wu_sb[:E, :1])
            if use_collectives:
                nc.gpsimd.collective_compute(
                    "AllReduce", mybir.AluOpType.add, replica_groups=rg,
                    ins=[cc_din.opt()], outs=[cc_dout.opt()],
                )

            # ---- parameter loads (scalar HWDGE ring; sync ring is A0's).
            # xw first: the stream matmuls need its leading slices ASAP.
            xw_sb = xwp.tile([128, NKT * E], bf16, name="xw_sb")
            for s in range(8):
                w = NKT * E // 8
                nc.scalar.dma_start(xw_sb[:, s * w:(s + 1) * w],
                                    xwt[:, s * w:(s + 1) * w])
            w2_sb = constp.tile([E, R * C], bf16, name="w2_sb")
            nc.scalar.dma_start(w2_sb[:], w2a[:])
            b1_sb = constp.tile([E, 1], f32, name="b1_sb")
            nc.scalar.dma_start(b1_sb[:], b1[:])
            b2_sb = constp.tile([C, 1], f32, name="b2_sb")
            nc.scalar.dma_start(b2_sb[:], b2[:])
            for rep in range(repeats):
                # collective bounce buffers
                cc_in = dramp.tile([E, N1], bf16, name=f"cc_in{rep}")
                cc_out = dramp.tile([E, N1], bf16, name=f"cc_out{rep}",
                                    addr_space="Shared")

                # ---- stream phase: h1.T partial accumulates in two PSUM
                # banks (n1 halves).
                ps_h1a = psh1.tile([E, 512], f32, name="ps_h1a", tag="ps_a")
                ps_h1b = psh1.tile([E, 512], f32, name="ps_h1b", tag="ps_b")
                last_mm = None
                for g in range(NG):
                    a0_sb = a0p.tile([128, KT_PER_DMA * N1], bf16,
                                     name="a0_sb", tag="a0")
                    base = g * KT_PER_DMA * N1
                    nc.sync.dma_start(a0_sb[:],
                                      a0t[:, base:base + KT_PER_DMA * N1])
                    for i in range(KT_PER_DMA):
                        kt = g * KT_PER_DMA + i
                        nc.tensor.matmul(
                            ps_h1a[:],
                            xw_sb[:, kt * E:(kt + 1) * E],
                            a0_sb[:, i * N1:i * N1 + 512],
                            start=(kt == 0), stop=(kt == NKT - 1),
                            skip_group_check=True,
                        )
                        last_mm = nc.tensor.matmul(
                            ps_h1b[:],
                            xw_sb[:, kt * E:(kt + 1) * E],
                            a0_sb[:, i * N1 + 512:(i + 1) * N1],
                            start=(kt == 0), stop=(kt == NKT - 1),
                            skip_group_check=True,
                        )

                # h1 partial -> bf16 SBUF -> DRAM bounce -> AllReduce
                h1part = h1p.tile([E, N1], bf16, name="h1part", tag="h1part")
                nc.vector.tensor_copy(h1part[:, :512], ps_h1a[:])
                nc.vector.tensor_copy(h1part[:, 512:], ps_h1b[:])
                nc.scalar.dma_start(cc_in[:], h1part[:])
                # a1 is only needed in the post phase; loading it here keeps
                # the early HBM window clear for A0/xw and lands during the
                # AllReduce wait.
                a1_sb = a1p.tile([128, NKT * C], bf16, name="a1_sb")
                nc.scalar.dma_start(a1_sb[:], a1t[:])
                if use_collectives:
                    nc.gpsimd.collective_compute(
                        "AllReduce", mybir.AluOpType.add, replica_groups=rg,
                        ins=[cc_in.opt()], outs=[cc_out.opt()],
                    )
                else:  # single-core timing variant
                    nc.gpsimd.dma_start(cc_out[:], cc_in[:])

                # ---- keep-warm matmuls spanning the AllReduce wait ----
                ps_wu2 = pswu.tile([128, 512], f32, name="ps_wu2",
                                   tag="ps_wu")
                for i in range(WARM2_MMS):
                    mm = nc.tensor.matmul(
                        ps_wu2[:], wu_sb[:, :128], wu_sb[:],
                        start=(i == 0), stop=(i == WARM2_MMS - 1),
                    )
                    if i == 0 and last_mm is not None:
                        tile_mod.add_dep_helper(
                            mm.ins, last_mm.ins, sync=False,
                            reason="keep-warm after stream")

                # ---- post phase: relu + layer 2 + out accumulation ----
                h1s = h1p.tile([E, N1], bf16, name="h1s", tag="h1s")
                nc.scalar.dma_start(h1s[:], cc_out[:])
                h1r = h1p.tile([E, N1], bf16, name="h1r", tag="h1r")
                nc.scalar.activation(
                    h1r[:], h1s[:],
                    mybir.ActivationFunctionType.Relu,
                    bias=b1_sb[:],
                )

                ps_out = psoutp.tile([C, NOPC], f32, name="ps_out",
                                     tag="ps_out")
                h2ts = {}
                for b in range(NB1):
                    ps2 = psh2.tile([128, R * C], f32, name="ps_h2",
                                    tag="ps_h2")
                    nc.tensor.matmul(
                        ps2[:], h1r[:, b * 128:(b + 1) * 128], w2_sb[:],
                        start=True, stop=True,
                    )
                    h2t = h2p.tile([128, R * C], bf16, name="h2t", tag="h2t")
                    if b % 2 == 0:
                        nc.vector.tensor_copy(h2t[:], ps2[:])
                    else:
                        nc.scalar.copy(h2t[:], ps2[:])
                    h2ts[b] = h2t

                nfinal = R * NB1
                ifinal = 0
                for b in range(NB1):
                    for r in range(R):
                        t = r * NB1 + b
                        nc.tensor.matmul(
                            ps_out[:],
                            h2ts[b][:, r * C:(r + 1) * C],
                            a1_sb[:, t * C:(t + 1) * C],
                            start=(ifinal == 0),
                            stop=(ifinal == nfinal - 1),
                            skip_group_check=True,
                        )
                        ifinal += 1

                # ---- bias2 + store this core's out.T slice ----
                out_sb = constp.tile([C, NOPC], f32, name="out_sb",
                                     tag="out_sb")
                nc.vector.tensor_scalar_add(out_sb[:], ps_out[:], b2_sb[:])
                nc.gpsimd.dma_start(outT[:], out_sb[:])


    nc.compile()
    return nc


def _get_module():
    if "nc" not in _cache:
        _cache["nc"] = _build_module()
    return _cache["nc"]


def make_in_maps(X_batch, sel_idx, A0, A1, comp1, bases1, comp2, bases2,
                 bias1, bias2):
    """Host-side sharding / layout prep -> per-core input maps."""
    X_batch = np.asarray(X_batch, dtype=np.float32)
    sel_idx = np.asarray(sel_idx)
    A0 = np.asarray(A0, dtype=np.float32)
    A1 = np.asarray(A1, dtype=np.float32)
    comp1 = np.asarray(comp1, dtype=np.float32)
    bases1 = np.asarray(bases1, dtype=np.float32)
    comp2 = np.asarray(comp2, dtype=np.float32)
    bases2 = np.asarray(bases2, dtype=np.float32)
    bias1 = np.asarray(bias1, dtype=np.float32)
    bias2 = np.asarray(bias2, dtype=np.float32)

    bf = ml_dtypes.bfloat16

    x = X_batch[sel_idx.astype(np.int64)]                    # [N2, F]

    w1 = np.einsum("rb,bfe->rfe", comp1, bases1)             # [R, F, E]
    w2 = np.einsum("rb,bec->rec", comp2, bases2)             # [R, E, C]
    w2a_host = np.ascontiguousarray(
        w2.transpose(1, 0, 2).reshape(E, R * C).astype(bf))  # [E, R*C]

    A0b = A0.astype(bf)                                      # [N1, R*N2]
    a1T = np.ascontiguousarray(A1.astype(bf).T)              # [R*N1, NOUT]

    b1_host = np.ascontiguousarray(bias1.reshape(E, 1))
    b2_host = np.ascontiguousarray(bias2.reshape(C, 1))

    in_maps = []
    for c in range(NCORES):
        # xw for this core's RPC relations (the sharding hint's "matching
        # relation slices of xw"), packed so k-tile kt = nb*RPC + rl lives at
        # xwt[:, kt*E:(kt+1)*E] with the k-rows on partitions.
        xw = np.einsum("nf,rfe->rne", x,
                       w1[RPC * c:RPC * (c + 1)])            # [RPC, N2, E]
        xw_pack = np.ascontiguousarray(
            xw.reshape(RPC, NB2, 128, E)                     # rl nb p e
              .transpose(2, 1, 0, 3)                         # p nb rl e
              .reshape(128, NKT * E).astype(bf))
        # A0 pack: pack[p, kt*N1 + n] = A0[n, c*KPC + rl*N2 + nb*128 + p]
        # with kt = nb*RPC + rl  (nb-major to match xw_pack).
        blk = A0b[:, c * KPC:(c + 1) * KPC]                  # [N1, KPC]
        a0_pack = np.ascontiguousarray(
            blk.reshape(N1, RPC, NB2, 128)                   # n rl nb p
               .transpose(3, 2, 1, 0)                        # p nb rl n
               .reshape(128, NKT * N1))
        # core c's 32 output rows: pack A1.T[:, 32c:32c+32] so each k-tile is
        # a [128, 32] slice living at a1t[:, t*32:(t+1)*32]
        a1_blk = a1T[:, NOPC * c:NOPC * (c + 1)]             # [R*N1, 32]
        a1_pack = np.ascontiguousarray(
            a1_blk.reshape(NKT, 128, C).transpose(1, 0, 2).reshape(128,
                                                                   NKT * C))
        in_maps.append({
            "xwt": xw_pack,
            "a0t": a0_pack,
            "a1t": a1_pack,
            "w2a": w2a_host,
            "b1": b1_host,
            "b2": b2_host,
        })
    return in_maps


def kernel(X_batch, sel_idx, A0, A1, comp1, bases1, comp2, bases2,
           bias1, bias2):
    global last_results
    from concourse.bass_utils import run_bass_kernel_spmd

    in_maps = make_in_maps(X_batch, sel_idx, A0, A1, comp1, bases1,
                           comp2, bases2, bias1, bias2)
    nc = _get_module()
    res = run_bass_kernel_spmd(nc, in_maps, core_ids=list(range(NCORES)))
    last_results = res

    outT = np.concatenate([res.results[c]["outT"] for c in range(NCORES)],
                          axis=1)                            # [C, NOUT]
    return np.ascontiguousarray(outT.T)                      # [NOUT, C]


# revision 9
# speedup vs baseline: 1.0637x; 1.0637x over previous
"""Trainium2 Bass kernel: LADIES mini-batch ER-GCN (2-layer relational GCN).

Contract: kernel(**inputs) takes the FULL unsharded inputs (numpy, keyed as in
setup_inputs) and returns the FULL [256, 32] float32 output.

Strategy (8 NeuronCores, relation-sharded layer 1, output-row-sharded layer 2):
  - h1 = relu(A0 @ xw + b1) dominates: A0 is [1024, 131072] f32 = 512 MB.
    Core c owns relations {2c, 2c+1} = a contiguous 64 MB column block of A0,
    host-packed into the exact [128-partition, k-tile, n1] order the kernel
    consumes and streamed as bf16 (32 MB/core): every 1 MB DMA is 8 KB-
    contiguous per partition (full HBM rate).
  - The tiny per-relation xw = x @ w1[r] slices (2 MB/core, the same bytes as
    shipping x itself) are precomputed on the host per the sharding hint
    ("split A_0 column-blocks and the matching relation slices of xw"), so
    the PE only runs the A0-stream matmuls and never stalls on an
    xt -> xw -> copy chain.
  - A dep-free warm-up matmul block at t~0 lifts HAM out of the K=4/8
    half-rate state before the stream begins; an anchored second block keeps
    the PE warm across the AllReduce wait so the layer-2 matmuls run warm.
  - ONE AllReduce of the full h1 partial [64, 1024] in bf16 (128 KB).  A
    dep-free dummy 1-column AllReduce issued at t~0 absorbs the ~11.5 us
    first-collective setup and queues behind the runtime's entry barrier, so
    the real AR starts ~1 us after its doorbell.
  - Layer 2 runs fully in bf16: after the AR every core has full h1; core c
    computes out.T[:, 32c:32c+32] (its 32 output rows) against a host-packed
    A1.T column block -- no second collective; the host concatenates slices.
"""

import numpy as np
import ml_dtypes

# Problem dimensions (fixed by the problem spec).
R, NB = 16, 16
N2, N1, NOUT = 8192, 1024, 256
F, E, C = 128, 64, 32

NCORES = 8
RPC = R // NCORES            # relations per core = 2
KPC = RPC * N2               # layer-1 contraction rows per core = 16384
NKT = KPC // 128             # k-tiles per core = 128
NB2 = N2 // 128              # n2-blocks per relation = 64
NB1 = N1 // 128              # n1-blocks = 8
KT_PER_DMA = 4               # k-tiles per A0 DMA (1 MB bf16 transfers)
NG = NKT // KT_PER_DMA       # A0 DMA groups = 32
NOPC = NOUT // NCORES        # output rows per core = 32
WARM_MMS = 36                # dep-free warm-up: bridges until A0 group 0 lands
WARM2_MMS = 90               # keep-warm matmuls across the AllReduce wait

_cache = {}
last_results = None          # BassKernelResults from the most recent run


def _build_module(repeats=1, use_collectives=True):
    import concourse.bacc as bacc
    import concourse.tile as tile
    import concourse.tile as tile_mod
    import concourse.mybir as mybir

    f32 = mybir.dt.float32
    bf16 = mybir.dt.bfloat16

    nc = bacc.Bacc("TRN2", target_bir_lowering=False, debug=False,
                   num_devices=NCORES)

    xwt = nc.dram_tensor("xwt", [128, NKT * E], bf16, kind="ExternalInput")
    a0t = nc.dram_tensor("a0t", [128, NKT * N1], bf16, kind="ExternalInput")
    a1t = nc.dram_tensor("a1t", [128, NKT * C], bf16, kind="ExternalInput")
    w2a = nc.dram_tensor("w2a", [E, R * C], bf16, kind="ExternalInput")
    b1 = nc.dram_tensor("b1", [E, 1], f32, kind="ExternalInput")
    b2 = nc.dram_tensor("b2", [C, 1], f32, kind="ExternalInput")
    outT = nc.dram_tensor("outT", [C, NOPC], f32, kind="ExternalOutput")

    rg = [list(range(NCORES))]

    with tile.TileContext(nc) as tc:
        with (
            tc.tile_pool(name="const", bufs=1) as constp,
            tc.tile_pool(name="xwp", bufs=1) as xwp,
            tc.tile_pool(name="a0p", bufs=14) as a0p,
            tc.tile_pool(name="a1p", bufs=1) as a1p,
            tc.tile_pool(name="h1p", bufs=2) as h1p,
            tc.tile_pool(name="h2p", bufs=8) as h2p,
            tc.tile_pool(name="pswu", bufs=1, space="PSUM") as pswu,
            tc.tile_pool(name="psh1", bufs=1, space="PSUM") as psh1,
            tc.tile_pool(name="psh2", bufs=3, space="PSUM") as psh2,
            tc.tile_pool(name="psout", bufs=1, space="PSUM") as psoutp,
            tc.tile_pool(name="dram", bufs=1, space="DRAM") as dramp,
        ):
            # ---- PE warm-up: dep-free back-to-back matmuls so HAM reaches
            # K=8/8 before the stream matmuls begin.
            wu_sb = constp.tile([128, 512], bf16, name="wu_sb")
            nc.vector.memset(wu_sb[:], 0.0)
            ps_wu = pswu.tile([128, 512], f32, name="ps_wu", tag="ps_wu")
            for i in range(WARM_MMS):
                nc.tensor.matmul(
                    ps_wu[:], wu_sb[:, :128], wu_sb[:],
                    start=(i == 0), stop=(i == WARM_MMS - 1),
                )

            # ---- dummy first collective: absorbs the ~11.5 us first-AR
            # ncfw setup + entry barrier off the critical path.
            cc_din = dramp.tile([E, 1], bf16, name="cc_din")
            cc_dout = dramp.tile([E, 1], bf16, name="cc_dout",
                                 addr_space="Shared")
            nc.gpsimd.dma_start(cc_din[:], wu_sb[:E, :1])
            if use_collectives:
                nc.gpsimd.collective_compute(
                    "AllReduce", mybir.AluOpType.add, replica_groups=rg,
                    ins=[cc_din.opt()], outs=[cc_dout.opt()],
                )

            # ---- parameter loads (scalar HWDGE ring; sync ring is A0's).
            # xw first: the stream matmuls need its leading slices ASAP.
            xw_sb = xwp.tile([128, NKT * E], bf16, name="xw_sb")
            for s in range(8):
                w = NKT * E // 8
                nc.scalar.dma_start(xw_sb[:, s * w:(s + 1) * w],
                                    xwt[:, s * w:(s + 1) * w])
            w2_sb = constp.tile([E, R * C], bf16, name="w2_sb")
            nc.scalar.dma_start(w2_sb[:], w2a[:])
            b1_sb = constp.tile([E, 1], f32, name="b1_sb")
            nc.scalar.dma_start(b1_sb[:], b1[:])
            b2_sb = constp.tile([C, 1], f32, name="b2_sb")
            nc.scalar.dma_start(b2_sb[:], b2[:])
            for rep in range(repeats):
                # collective bounce buffers
                cc_in = dramp.tile([E, N1], bf16, name=f"cc_in{rep}")
                cc_out = dramp.tile([E, N1], bf16, name=f"cc_out{rep}",
                                    addr_space="Shared")

                # ---- stream phase: h1.T partial accumulates in two PSUM
                # banks (n1 halves).
                ps_h1a = psh1.tile([E, 512], f32, name="ps_h1a", tag="ps_a")
                ps_h1b = psh1.tile([E, 512], f32, name="ps_h1b", tag="ps_b")
                last_mm = None
                for g in range(NG):
                    a0_sb = a0p.tile([128, KT_PER_DMA * N1], bf16,
                                     name="a0_sb", tag="a0")
                    base = g * KT_PER_DMA * N1
                    nc.sync.dma_start(a0_sb[:],
                                      a0t[:, base:base + KT_PER_DMA * N1])
                    for i in range(KT_PER_DMA):
                        kt = g * KT_PER_DMA + i
                        nc.tensor.matmul(
                            ps_h1a[:],
                            xw_sb[:, kt * E:(kt + 1) * E],
                            a0_sb[:, i * N1:i * N1 + 512],
                            start=(kt == 0), stop=(kt == NKT - 1),
                            skip_group_check=True,
                        )
                        last_mm = nc.tensor.matmul(
                            ps_h1b[:],
                            xw_sb[:, kt * E:(kt + 1) * E],
                            a0_sb[:, i * N1 + 512:(i + 1) * N1],
                            start=(kt == 0), stop=(kt == NKT - 1),
                            skip_group_check=True,
                        )

                # h1 partial -> bf16 SBUF -> DRAM bounce -> AllReduce
                h1part = h1p.tile([E, N1], bf16, name="h1part", tag="h1part")
                nc.vector.tensor_copy(h1part[:, :512], ps_h1a[:])
                nc.vector.tensor_copy(h1part[:, 512:], ps_h1b[:])
                nc.scalar.dma_start(cc_in[:], h1part[:])
                # a1 is only needed in the post phase; loading it here keeps
                # the early HBM window clear for A0/xw and lands during the
                # AllReduce wait.
                a1_sb = a1p.tile([128, NKT * C], bf16, name="a1_sb")
                nc.scalar.dma_start(a1_sb[:], a1t[:])
                if use_collectives:
                    nc.gpsimd.collective_compute(
                        "AllReduce", mybir.AluOpType.add, replica_groups=rg,
                        ins=[cc_in.opt()], outs=[cc_out.opt()],
                    )
                else:  # single-core timing variant
                    nc.gpsimd.dma_start(cc_out[:], cc_in[:])

                # ---- keep-warm matmuls spanning the AllReduce wait ----
                ps_wu2 = pswu.tile([128, 512], f32, name="ps_wu2",
                                   tag="ps_wu")
                for i in range(WARM2_MMS):
                    mm = nc.tensor.matmul(
                        ps_wu2[:], wu_sb[:, :128], wu_sb[:],
                        start=(i == 0), stop=(i == WARM2_MMS - 1),
                    )
                    if i == 0 and last_mm is not None:
                        tile_mod.add_dep_helper(
                            mm.ins, last_mm.ins, sync=False,
                            reason="keep-warm after stream")

                # ---- post phase: relu + layer 2 + out accumulation ----
                h1s = h1p.tile([E, N1], bf16, name="h1s", tag="h1s")
                nc.scalar.dma_start(h1s[:], cc_out[:])
                h1r = h1p.tile([E, N1], bf16, name="h1r", tag="h1r")
                nc.scalar.activation(
                    h1r[:], h1s[:],
                    mybir.ActivationFunctionType.Relu,
                    bias=b1_sb[:],
                )

                ps_out = psoutp.tile([C, NOPC], f32, name="ps_out",
                                     tag="ps_out")
                h2ts = {}
                for b in range(NB1):
                    ps2 = psh2.tile([128, R * C], f32, name="ps_h2",
                                    tag="ps_h2")
                    nc.tensor.matmul(
                        ps2[:], h1r[:, b * 128:(b + 1) * 128], w2_sb[:],
                        start=True, stop=True,
                    )
                    h2t = h2p.tile([128, R * C], bf16, name="h2t", tag="h2t")
                    if b % 2 == 0:
                        nc.vector.tensor_copy(h2t[:], ps2[:])
                    else:
                        nc.scalar.copy(h2t[:], ps2[:])
                    h2ts[b] = h2t

                nfinal = R * NB1
                ifinal = 0
                for b in range(NB1):
                    for r in range(R):
                        t = r * NB1 + b
                        nc.tensor.matmul(
                            ps_out[:],
                            h2ts[b][:, r * C:(r + 1) * C],
                            a1_sb[:, t * C:(t + 1) * C],
                            start=(ifinal == 0),
                            stop=(ifinal == nfinal - 1),
                            skip_group_check=True,
                        )
                        ifinal += 1

                # ---- bias2 + store this core's out.T slice ----
                out_sb = constp.tile([C, NOPC], f32, name="out_sb",
                                     tag="out_sb")
                nc.vector.tensor_scalar_add(out_sb[:], ps_out[:], b2_sb[:])
                nc.gpsimd.dma_start(outT[:], out_sb[:])


    nc.compile()
    return nc


def _get_module():
    if "nc" not in _cache:
        _cache["nc"] = _build_module()
    return _cache["nc"]


def make_in_maps(X_batch, sel_idx, A0, A1, comp1, bases1, comp2, bases2,
                 bias1, bias2):
    """Host-side sharding / layout prep -> per-core input maps."""
    X_batch = np.asarray(X_batch, dtype=np.float32)
    sel_idx = np.asarray(sel_idx)
    A0 = np.asarray(A0, dtype=np.float32)
    A1 = np.asarray(A1, dtype=np.float32)
    comp1 = np.asarray(comp1, dtype=np.float32)
    bases1 = np.asarray(bases1, dtype=np.float32)
    comp2 = np.asarray(comp2, dtype=np.float32)
    bases2 = np.asarray(bases2, dtype=np.float32)
    bias1 = np.asarray(bias1, dtype=np.float32)
    bias2 = np.asarray(bias2, dtype=np.float32)

    bf = ml_dtypes.bfloat16

    x = X_batch[sel_idx.astype(np.int64)]                    # [N2, F]

    w1 = np.einsum("rb,bfe->rfe", comp1, bases1)             # [R, F, E]
    w2 = np.einsum("rb,bec->rec", comp2, bases2)             # [R, E, C]
    w2a_host = np.ascontiguousarray(
        w2.transpose(1, 0, 2).reshape(E, R * C).astype(bf))  # [E, R*C]

    A0b = A0.astype(bf)                                      # [N1, R*N2]
    a1T = np.ascontiguousarray(A1.astype(bf).T)              # [R*N1, NOUT]

    b1_host = np.ascontiguousarray(bias1.reshape(E, 1))
    b2_host = np.ascontiguousarray(bias2.reshape(C, 1))

    in_maps = []
    for c in range(NCORES):
        # xw for this core's RPC relations (the sharding hint's "matching
        # relation slices of xw"), packed so k-tile kt = nb*RPC + rl lives at
        # xwt[:, kt*E:(kt+1)*E] with the k-rows on partitions.
        xw = np.einsum("nf,rfe->rne", x,
                       w1[RPC * c:RPC * (c + 1)])            # [RPC, N2, E]
        xw_pack = np.ascontiguousarray(
            xw.reshape(RPC, NB2, 128, E)                     # rl nb p e
              .transpose(2, 1, 0, 3)                         # p nb rl e
              .reshape(128, NKT * E).astype(bf))
        # A0 pack: pack[p, kt*N1 + n] = A0[n, c*KPC + rl*N2 + nb*128 + p]
        # with kt = nb*RPC + rl  (nb-major to match xw_pack).
        blk = A0b[:, c * KPC:(c + 1) * KPC]                  # [N1, KPC]
        a0_pack = np.ascontiguousarray(
            blk.reshape(N1, RPC, NB2, 128)                   # n rl nb p
               .transpose(3, 2, 1, 0)                        # p nb rl n
               .reshape(128, NKT * N1))
        # core c's 32 output rows: pack A1.T[:, 32c:32c+32] so each k-tile is
        # a [128, 32] slice living at a1t[:, t*32:(t+1)*32]
        a1_blk = a1T[:, NOPC * c:NOPC * (c + 1)]             # [R*N1, 32]
        a1_pack = np.ascontiguousarray(
            a1_blk.reshape(NKT, 128, C).transpose(1, 0, 2).reshape(128,
                                                                   NKT * C))
        in_maps.append({
            "xwt": xw_pack,
            "a0t": a0_pack,
            "a1t": a1_pack,
            "w2a": w2a_host,
            "b1": b1_host,
            "b2": b2_host,
        })
    return in_maps


def kernel(X_batch, sel_idx, A0, A1, comp1, bases1, comp2, bases2,
           bias1, bias2):
    global last_results
    from concourse.bass_utils import run_bass_kernel_spmd

    in_maps = make_in_maps(X_batch, sel_idx, A0, A1, comp1, bases1,
                           comp2, bases2, bias1, bias2)
    nc = _get_module()
    res = run_bass_kernel_spmd(nc, in_maps, core_ids=list(range(NCORES)))
    last_results = res

    outT = np.concatenate([res.results[c]["outT"] for c in range(NCORES)],
                          axis=1)                            # [C, NOUT]
    return np.ascontiguousarray(outT.T)                      # [NOUT, C]


# revision 10
# speedup vs baseline: 1.1104x; 1.0439x over previous
"""Trainium2 Bass kernel: LADIES mini-batch ER-GCN (2-layer relational GCN).

Contract: kernel(**inputs) takes the FULL unsharded inputs (numpy, keyed as in
setup_inputs) and returns the FULL [256, 32] float32 output.

Strategy (8 NeuronCores, relation-sharded layer 1, output-row-sharded layer 2):
  - h1 = relu(A0 @ xw + b1) dominates: A0 is [1024, 131072] f32 = 512 MB.
    Core c owns relations {2c, 2c+1} = a contiguous 64 MB column block of A0,
    host-packed into the exact [128-partition, k-tile, n1] order the kernel
    consumes and streamed as bf16 (32 MB/core): every 1 MB DMA is 8 KB-
    contiguous per partition (full HBM rate).
  - The tiny per-relation xw = x @ w1[r] slices (2 MB/core, the same bytes as
    shipping x itself) are precomputed on the host per the sharding hint
    ("split A_0 column-blocks and the matching relation slices of xw"), so
    the PE only runs the A0-stream matmuls and never stalls on an
    xt -> xw -> copy chain.
  - A dep-free warm-up matmul block at t~0 lifts HAM out of the K=4/8
    half-rate state before the stream begins; an anchored second block keeps
    the PE warm across the AllReduce wait so the layer-2 matmuls run warm.
  - ONE AllReduce of the full h1 partial [64, 1024] in bf16 (128 KB).  A
    dep-free dummy 1-column AllReduce issued at t~0 absorbs the ~11.5 us
    first-collective setup and queues behind the runtime's entry barrier, so
    the real AR starts ~1 us after its doorbell.
  - Layer 2 runs fully in bf16: after the AR every core has full h1; core c
    computes out.T[:, 32c:32c+32] (its 32 output rows) against a host-packed
    A1.T column block -- no second collective; the host concatenates slices.
"""

import numpy as np
import ml_dtypes

# Problem dimensions (fixed by the problem spec).
R, NB = 16, 16
N2, N1, NOUT = 8192, 1024, 256
F, E, C = 128, 64, 32

NCORES = 8
RPC = R // NCORES            # relations per core = 2
KPC = RPC * N2               # layer-1 contraction rows per core = 16384
NKT = KPC // 128             # k-tiles per core = 128
NB2 = N2 // 128              # n2-blocks per relation = 64
NB1 = N1 // 128              # n1-blocks = 8
KT_PER_DMA = 4               # k-tiles per A0 DMA (1 MB bf16 transfers)
NG = NKT // KT_PER_DMA       # A0 DMA groups = 32
NOPC = NOUT // NCORES        # output rows per core = 32
WARM_MMS = 36                # dep-free warm-up: bridges until A0 group 0 lands
WARM2_MMS = 90               # keep-warm matmuls across the AllReduce wait

_cache = {}
last_results = None          # BassKernelResults from the most recent run


def _build_module(repeats=1, use_collectives=True):
    import concourse.bacc as bacc
    import concourse.tile as tile
    import concourse.tile as tile_mod
    import concourse.mybir as mybir

    f32 = mybir.dt.float32
    bf16 = mybir.dt.bfloat16

    nc = bacc.Bacc("TRN2", target_bir_lowering=False, debug=False,
                   num_devices=NCORES)

    xwt = nc.dram_tensor("xwt", [128, NKT * E], bf16, kind="ExternalInput")
    a0t = nc.dram_tensor("a0t", [128, NKT * N1], bf16, kind="ExternalInput")
    a1t = nc.dram_tensor("a1t", [128, NKT * C], bf16, kind="ExternalInput")
    w2a = nc.dram_tensor("w2a", [E, R * C], bf16, kind="ExternalInput")
    b1 = nc.dram_tensor("b1", [E, 1], f32, kind="ExternalInput")
    b2 = nc.dram_tensor("b2", [C, 1], f32, kind="ExternalInput")
    outT = nc.dram_tensor("outT", [C, NOPC], f32, kind="ExternalOutput")

    rg = [list(range(NCORES))]

    with tile.TileContext(nc) as tc:
        with (
            tc.tile_pool(name="const", bufs=1) as constp,
            tc.tile_pool(name="xwp", bufs=1) as xwp,
            tc.tile_pool(name="a0p", bufs=14) as a0p,
            tc.tile_pool(name="a1p", bufs=1) as a1p,
            tc.tile_pool(name="h1p", bufs=2) as h1p,
            tc.tile_pool(name="h2p", bufs=8) as h2p,
            tc.tile_pool(name="pswu", bufs=1, space="PSUM") as pswu,
            tc.tile_pool(name="psh1", bufs=1, space="PSUM") as psh1,
            tc.tile_pool(name="psh2", bufs=3, space="PSUM") as psh2,
            tc.tile_pool(name="psout", bufs=1, space="PSUM") as psoutp,
            tc.tile_pool(name="dram", bufs=1, space="DRAM") as dramp,
        ):
            # ---- PE warm-up: dep-free back-to-back matmuls so HAM reaches
            # K=8/8 before the stream matmuls begin.
            wu_sb = constp.tile([128, 512], bf16, name="wu_sb")
            nc.vector.memset(wu_sb[:], 0.0)
            ps_wu = pswu.tile([128, 512], f32, name="ps_wu", tag="ps_wu")
            for i in range(WARM_MMS):
                nc.tensor.matmul(
                    ps_wu[:], wu_sb[:, :128], wu_sb[:],
                    start=(i == 0), stop=(i == WARM_MMS - 1),
                )

            # ---- parameter loads (scalar HWDGE ring; sync ring is A0's).
            # xw first: the stream matmuls need its leading slices ASAP.
            xw_sb = xwp.tile([128, NKT * E], bf16, name="xw_sb")
            for s in range(8):
                w = NKT * E // 8
                nc.scalar.dma_start(xw_sb[:, s * w:(s + 1) * w],
                                    xwt[:, s * w:(s + 1) * w])
            w2_sb = constp.tile([E, R * C], bf16, name="w2_sb")
            nc.scalar.dma_start(w2_sb[:], w2a[:])
            b1_sb = constp.tile([E, 1], f32, name="b1_sb")
            nc.scalar.dma_start(b1_sb[:], b1[:])
            b2_sb = constp.tile([C, 1], f32, name="b2_sb")
            nc.scalar.dma_start(b2_sb[:], b2[:])
            for rep in range(repeats):
                # collective bounce buffers
                cc_in = dramp.tile([E, N1], bf16, name=f"cc_in{rep}")
                cc_out = dramp.tile([E, N1], bf16, name=f"cc_out{rep}",
                                    addr_space="Shared")

                # ---- stream phase: h1.T partial accumulates in two PSUM
                # banks (n1 halves).
                ps_h1a = psh1.tile([E, 512], f32, name="ps_h1a", tag="ps_a")
                ps_h1b = psh1.tile([E, 512], f32, name="ps_h1b", tag="ps_b")
                last_mm = None
                for g in range(NG):
                    a0_sb = a0p.tile([128, KT_PER_DMA * N1], bf16,
                                     name="a0_sb", tag="a0")
                    base = g * KT_PER_DMA * N1
                    nc.sync.dma_start(a0_sb[:],
                                      a0t[:, base:base + KT_PER_DMA * N1])
                    for i in range(KT_PER_DMA):
                        kt = g * KT_PER_DMA + i
                        nc.tensor.matmul(
                            ps_h1a[:],
                            xw_sb[:, kt * E:(kt + 1) * E],
                            a0_sb[:, i * N1:i * N1 + 512],
                            start=(kt == 0), stop=(kt == NKT - 1),
                            skip_group_check=True,
                        )
                        last_mm = nc.tensor.matmul(
                            ps_h1b[:],
                            xw_sb[:, kt * E:(kt + 1) * E],
                            a0_sb[:, i * N1 + 512:(i + 1) * N1],
                            start=(kt == 0), stop=(kt == NKT - 1),
                            skip_group_check=True,
                        )

                # h1 partial -> bf16 SBUF -> DRAM bounce -> AllReduce
                h1part = h1p.tile([E, N1], bf16, name="h1part", tag="h1part")
                nc.vector.tensor_copy(h1part[:, :512], ps_h1a[:])
                nc.vector.tensor_copy(h1part[:, 512:], ps_h1b[:])
                nc.scalar.dma_start(cc_in[:], h1part[:])
                # a1 is only needed in the post phase; loading it here keeps
                # the early HBM window clear for A0/xw and lands during the
                # AllReduce wait.
                a1_sb = a1p.tile([128, NKT * C], bf16, name="a1_sb")
                nc.scalar.dma_start(a1_sb[:], a1t[:])
                if use_collectives:
                    nc.gpsimd.collective_compute(
                        "AllReduce", mybir.AluOpType.add, replica_groups=rg,
                        ins=[cc_in.opt()], outs=[cc_out.opt()],
                    )
                else:  # single-core timing variant
                    nc.gpsimd.dma_start(cc_out[:], cc_in[:])

                # ---- keep-warm matmuls spanning the AllReduce wait ----
                ps_wu2 = pswu.tile([128, 512], f32, name="ps_wu2",
                                   tag="ps_wu")
                for i in range(WARM2_MMS):
                    mm = nc.tensor.matmul(
                        ps_wu2[:], wu_sb[:, :128], wu_sb[:],
                        start=(i == 0), stop=(i == WARM2_MMS - 1),
                    )
                    if i == 0 and last_mm is not None:
                        tile_mod.add_dep_helper(
                            mm.ins, last_mm.ins, sync=False,
                            reason="keep-warm after stream")

                # ---- post phase: relu + layer 2 + out accumulation ----
                h1s = h1p.tile([E, N1], bf16, name="h1s", tag="h1s")
                nc.scalar.dma_start(h1s[:], cc_out[:])
                h1r = h1p.tile([E, N1], bf16, name="h1r", tag="h1r")
                nc.scalar.activation(
                    h1r[:], h1s[:],
                    mybir.ActivationFunctionType.Relu,
                    bias=b1_sb[:],
                )

                ps_out = psoutp.tile([C, NOPC], f32, name="ps_out",
                                     tag="ps_out")
                h2ts = {}
                for b in range(NB1):
                    ps2 = psh2.tile([128, R * C], f32, name="ps_h2",
                                    tag="ps_h2")
                    nc.tensor.matmul(
                        ps2[:], h1r[:, b * 128:(b + 1) * 128], w2_sb[:],
                        start=True, stop=True,
                    )
                    h2t = h2p.tile([128, R * C], bf16, name="h2t", tag="h2t")
                    if b % 2 == 0:
                        nc.vector.tensor_copy(h2t[:], ps2[:])
                    else:
                        nc.scalar.copy(h2t[:], ps2[:])
                    h2ts[b] = h2t

                nfinal = R * NB1
                ifinal = 0
                for b in range(NB1):
                    for r in range(R):
                        t = r * NB1 + b
                        nc.tensor.matmul(
                            ps_out[:],
                            h2ts[b][:, r * C:(r + 1) * C],
                            a1_sb[:, t * C:(t + 1) * C],
                            start=(ifinal == 0),
                            stop=(ifinal == nfinal - 1),
                            skip_group_check=True,
                        )
                        ifinal += 1

                # ---- bias2 + store this core's out.T slice ----
                out_sb = constp.tile([C, NOPC], f32, name="out_sb",
                                     tag="out_sb")
                nc.vector.tensor_scalar_add(out_sb[:], ps_out[:], b2_sb[:])
                nc.gpsimd.dma_start(outT[:], out_sb[:])


    nc.compile()
    return nc


def _get_module():
    if "nc" not in _cache:
        _cache["nc"] = _build_module()
    return _cache["nc"]


def make_in_maps(X_batch, sel_idx, A0, A1, comp1, bases1, comp2, bases2,
                 bias1, bias2):
    """Host-side sharding / layout prep -> per-core input maps."""
    X_batch = np.asarray(X_batch, dtype=np.float32)
    sel_idx = np.asarray(sel_idx)
    A0 = np.asarray(A0, dtype=np.float32)
    A1 = np.asarray(A1, dtype=np.float32)
    comp1 = np.asarray(comp1, dtype=np.float32)
    bases1 = np.asarray(bases1, dtype=np.float32)
    comp2 = np.asarray(comp2, dtype=np.float32)
    bases2 = np.asarray(bases2, dtype=np.float32)
    bias1 = np.asarray(bias1, dtype=np.float32)
    bias2 = np.asarray(bias2, dtype=np.float32)

    bf = ml_dtypes.bfloat16

    x = X_batch[sel_idx.astype(np.int64)]                    # [N2, F]

    w1 = np.einsum("rb,bfe->rfe", comp1, bases1)             # [R, F, E]
    w2 = np.einsum("rb,bec->rec", comp2, bases2)             # [R, E, C]
    w2a_host = np.ascontiguousarray(
        w2.transpose(1, 0, 2).reshape(E, R * C).astype(bf))  # [E, R*C]

    A0b = A0.astype(bf)                                      # [N1, R*N2]
    a1T = np.ascontiguousarray(A1.astype(bf).T)              # [R*N1, NOUT]

    b1_host = np.ascontiguousarray(bias1.reshape(E, 1))
    b2_host = np.ascontiguousarray(bias2.reshape(C, 1))

    in_maps = []
    for c in range(NCORES):
        # xw for this core's RPC relations (the sharding hint's "matching
        # relation slices of xw"), packed so k-tile kt = nb*RPC + rl lives at
        # xwt[:, kt*E:(kt+1)*E] with the k-rows on partitions.
        xw = np.einsum("nf,rfe->rne", x,
                       w1[RPC * c:RPC * (c + 1)])            # [RPC, N2, E]
        xw_pack = np.ascontiguousarray(
            xw.reshape(RPC, NB2, 128, E)                     # rl nb p e
              .transpose(2, 1, 0, 3)                         # p nb rl e
              .reshape(128, NKT * E).astype(bf))
        # A0 pack: pack[p, kt*N1 + n] = A0[n, c*KPC + rl*N2 + nb*128 + p]
        # with kt = nb*RPC + rl  (nb-major to match xw_pack).
        blk = A0b[:, c * KPC:(c + 1) * KPC]                  # [N1, KPC]
        a0_pack = np.ascontiguousarray(
            blk.reshape(N1, RPC, NB2, 128)                   # n rl nb p
               .transpose(3, 2, 1, 0)                        # p nb rl n
               .reshape(128, NKT * N1))
        # core c's 32 output rows: pack A1.T[:, 32c:32c+32] so each k-tile is
        # a [128, 32] slice living at a1t[:, t*32:(t+1)*32]
        a1_blk = a1T[:, NOPC * c:NOPC * (c + 1)]             # [R*N1, 32]
        a1_pack = np.ascontiguousarray(
            a1_blk.reshape(NKT, 128, C).transpose(1, 0, 2).reshape(128,
                                                                   NKT * C))
        in_maps.append({
            "xwt": xw_pack,
            "a0t": a0_pack,
            "a1t": a1_pack,
            "w2a": w2a_host,
            "b1": b1_host,
            "b2": b2_host,
        })
    return in_maps


def kernel(X_batch, sel_idx, A0, A1, comp1, bases1, comp2, bases2,
           bias1, bias2):
    global last_results
    from concourse.bass_utils import run_bass_kernel_spmd

    in_maps = make_in_maps(X_batch, sel_idx, A0, A1, comp1, bases1,
                           comp2, bases2, bias1, bias2)
    nc = _get_module()
    res = run_bass_kernel_spmd(nc, in_maps, core_ids=list(range(NCORES)))
    last_results = res

    outT = np.concatenate([res.results[c]["outT"] for c in range(NCORES)],
                          axis=1)                            # [C, NOUT]
    return np.ascontiguousarray(outT.T)                      # [NOUT, C]
